# revision 11
# baseline (speedup 1.0000x reference)
"""CovPool kernel for 8 TRN2 NeuronCores.

reference semantics (B=32, N=16384, D=64):
    cov_b = (X_b - mean_b)^T (X_b - mean_b) / (N-1) + lam*I        (64x64)
    out   = sort(concat_b triu(cov_b)) reshaped to (B, 2080)

Device strategy (data parallel over batch):
  - core c owns batches [4c, 4c+4): streams its 16 MB slab once in
    256 KiB DMA chunks (2 KB contiguous per partition, ring of 16),
    alternating the sync/scalar HWDGE rings. Neither issuing engine
    does any work that waits on a stream DMA, so issuance never
    stalls (measured best among 128K-4M chunk sizes; DMA-bound at
    ~56 us/core vs the 46.9 us HBM roofline).
  - vector engine casts each chunk f32 -> bf16 into 65-column slices
    whose last column is a persistent 1.0 (written once at init). Each
    Gram matmul then uses lhsT = [data|1] (128, 65) so PSUM rows 0-63
    accumulate X^T X while row 64 accumulates the column sums -- no
    separate ones-matmuls and no fold.
  - per batch: 128 Gram matmuls accumulate, then gpsimd copies s out,
    a K=1 matmul adds -s s^T / N, DVE scales by 1/(N-1) and adds
    lam*I, gpsimd masks the strict lower triangle to +BIG and DMAs
    the tile out. The whole epilogue avoids the sync/scalar engines
    so stream-DMA issuance never queues behind an epilogue wait.
    Host extracts triu + sorts (tiny: 32 x 2080 values).
"""

import sys

sys.path.insert(0, "/opt/trn_rl_repo")

import numpy as np

from concourse import bacc, mybir
from concourse.tile import TileContext
from concourse.bass_utils import run_bass_kernel_spmd

B, N, D = 32, 16384, 64
NCORES = 8
BPC = B // NCORES  # batches per core
LAMBDA = 0.01
D_OUT = D * (D + 1) // 2  # 2080
BIG = 3.0e38  # lower-triangle fill (sorts above every real value)

CHUNK_ROWS = 1024  # x-rows per DMA chunk = 256 KiB f32
NSTREAM = 16  # stream ring depth
SLICE_W = D + 1  # 65 bf16 cols per slice: 64 data + persistent 1.0

f32 = mybir.dt.float32
bf16 = mybir.dt.bfloat16


def _emit_cov_body(tc, nc, x, out, stream, lam_tile, work_pool, psum_pool,
                   variant, ctr, chunk_rows, nstream, queues,
                   cast_engines=("vector",), split_dma=False):
    """One full covariance pass: stream all batches, write masked cov."""
    r_per_part = chunk_rows // 128
    dmas_per_batch = N // chunk_rows
    stream_f32, stream_bf = stream
    xf = x.rearrange("b n d -> b (n d)")  # flat per-batch view
    for b in range(BPC):
        psum = psum_pool.tile([D + 1, 512], f32, tag=f"acc{b % 4}")
        for t in range(dmas_per_batch):
            di = ctr["di"]
            buf = stream_f32[di % nstream]
            bbuf = stream_bf[di % nstream]
            eng = getattr(nc, queues[di % len(queues)])
            ctr["di"] = di + 1
            if variant != "mm_only":
                c0 = t * chunk_rows * D
                if split_dma:
                    # half-partition halves on both HWDGE rings at once:
                    # all 16 SBUF AXI ports busy per chunk, receipt
                    # latencies of the two rings overlap per-chunk
                    half = chunk_rows * D // 2
                    nc.sync.dma_start(
                        buf[0:64, :],
                        xf[b, c0:c0 + half].rearrange("(p f) -> p f", p=64),
                    )
                    nc.scalar.dma_start(
                        buf[64:128, :],
                        xf[b, c0 + half:c0 + 2 * half]
                        .rearrange("(p f) -> p f", p=64),
                    )
                else:
                    eng.dma_start(
                        buf[:],
                        xf[b, c0:c0 + chunk_rows * D]
                        .rearrange("(p f) -> p f", p=128),
                    )
            if variant == "dma_only":
                continue
            # f32 -> bf16 cast, strided into the 65-col slices
            # (col 64 of each slice keeps its init-time 1.0)
            src = buf[:].rearrange("p (r c) -> p r c", c=D)
            dst = bbuf[:].rearrange("p (r c) -> p r c", c=SLICE_W)[:, :, 0:D]
            ceng = getattr(nc, cast_engines[di % len(cast_engines)])
            ceng.tensor_scalar_mul(dst, src, 1.0)
            last_chunk = t == dmas_per_batch - 1
            for r in range(r_per_part):
                data = bbuf[:, r * SLICE_W:r * SLICE_W + D]
                if not (last_chunk and r == r_per_part - 1):
                    nc.tensor.matmul(
                        psum[0:D + 1, 0:D],
                        bbuf[:, r * SLICE_W:r * SLICE_W + SLICE_W],
                        data, start=(t == 0 and r == 0), stop=False,
                    )
                else:
                    # split the final slice so each PSUM region gets its
                    # own stop: the sum row closes here, the Gram rows
                    # stay open for the rank-1 mean correction.
                    nc.tensor.matmul(
                        psum[0:D, 0:D], data, data,
                        start=False, stop=False,
                    )
                    nc.tensor.matmul(
                        psum[D:D + 1, 0:D],
                        bbuf[:, r * SLICE_W + D:r * SLICE_W + SLICE_W],
                        data, start=False, stop=True,
                    )
        if variant == "dma_only":
            continue
        # epilogue: cov = (G - s s^T/N)/(N-1) + lam*I, mask, write out.
        # All on gpsimd/vector/tensor -- never sync/scalar, whose FIFOs
        # must stay free for stream-DMA issuance.
        s_sb = work_pool.tile([1, D], f32, tag="s_sb")
        s_neg = work_pool.tile([1, D], f32, tag="s_neg")
        nc.vector.tensor_scalar_mul(s_sb[:], psum[D:D + 1, 0:D], 1.0)
        nc.vector.tensor_scalar_mul(s_neg[:], s_sb[:], -1.0 / N)
        nc.tensor.matmul(
            psum[0:D, 0:D], s_sb[:], s_neg[:],
            start=False, stop=True,
        )
        cov_sb = work_pool.tile([D, D], f32, tag="cov")
        nc.vector.scalar_tensor_tensor(
            out=cov_sb[:], in0=psum[0:D, 0:D], scalar=1.0 / (N - 1),
            in1=lam_tile[:], op0=mybir.AluOpType.mult,
            op1=mybir.AluOpType.add,
        )
        # mask strict lower triangle (j < i) to BIG
        masked = work_pool.tile([D, D], f32, tag="masked")
        nc.gpsimd.affine_select(
            out=masked[:], in_=cov_sb[:], pattern=[[1, D]],
            compare_op=mybir.AluOpType.is_ge, fill=BIG,
            base=0, channel_multiplier=-1,
        )
        nc.gpsimd.dma_start(out[b], masked[:])


def build_cov_kernel(bench_reps=None, variant="full", unroll=1,
                     chunk_rows=CHUNK_ROWS, nstream=NSTREAM,
                     queues=("sync", "scalar"), cast_engines=("vector",),
                     split_dma=False):
    r_per_part = chunk_rows // 128
    chunk_f = chunk_rows * D // 128  # f32 per partition per chunk

    nc = bacc.Bacc("TRN2", target_bir_lowering=False, debug=False,
                   num_devices=NCORES)
    x = nc.dram_tensor("x", [BPC, N, D], f32, kind="ExternalInput")
    out = nc.dram_tensor("out", [BPC, D, D], f32, kind="ExternalOutput")

    with TileContext(nc) as tc:
        with (
            tc.tile_pool(name="stream", bufs=1) as stream_pool,
            tc.tile_pool(name="const", bufs=1) as const_pool,
            tc.tile_pool(name="work", bufs=2) as work_pool,
            tc.tile_pool(name="psum", bufs=1, space="PSUM") as psum_pool,
        ):
            # constants
            lam_tile = const_pool.tile([D, D], f32, tag="lam")
            nc.vector.memset(lam_tile[:], LAMBDA)
            # keep lam only on the diagonal: iota = j - i, keep where ==0
            nc.gpsimd.affine_select(
                out=lam_tile[:], in_=lam_tile[:], pattern=[[1, D]],
                compare_op=mybir.AluOpType.is_equal, fill=0.0,
                base=0, channel_multiplier=-1,
            )

            # stream ring: (128, chunk_f) f32 chunks + bf16 cast buffers
            # with a persistent 1.0 in col 64 of each 65-col slice
            # (memset once; casts never touch col 64)
            stream_f32 = [
                stream_pool.tile([128, chunk_f], f32,
                                 tag=f"stream{i}", name=f"stream{i}")
                for i in range(nstream)
            ]
            stream_bf = [
                stream_pool.tile([128, r_per_part * SLICE_W], bf16,
                                 tag=f"streambf{i}", name=f"streambf{i}")
                for i in range(nstream)
            ]
            for tbuf in stream_bf:
                nc.vector.memset(tbuf[:], 1.0)
            stream = (stream_f32, stream_bf)

            if variant == "mm_only":
                for tbuf in stream_f32:
                    nc.vector.memset(tbuf[:], 0.5)

            ctr = {"di": 0}

            def body():
                for _ in range(unroll):
                    _emit_cov_body(tc, nc, x, out, stream, lam_tile,
                                   work_pool, psum_pool, variant, ctr,
                                   chunk_rows, nstream, queues,
                                   cast_engines, split_dma)
                if variant == "dma_only":
                    # consume the stream buffers so Tile sees a reader,
                    # and produce the declared output
                    scrap = work_pool.tile([128, 1], f32, tag="scrap")
                    for tbuf in stream_f32:
                        nc.vector.tensor_reduce(
                            out=scrap[:], in_=tbuf[:],
                            axis=mybir.AxisListType.X,
                            op=mybir.AluOpType.max,
                        )
                    for b in range(BPC):
                        nc.sync.dma_start(out[b], stream_f32[0][0:D, 0:D])

            if bench_reps is None:
                body()
            else:
                with tc.For_i(0, bench_reps, 1):
                    body()

    nc.compile()
    return nc


_NC_CACHE = {}


def _get_kernel():
    if "nc" not in _NC_CACHE:
        _NC_CACHE["nc"] = build_cov_kernel()
    return _NC_CACHE["nc"]


def run_device(x_full: np.ndarray):
    """Run the bass kernel on 8 cores; returns per-core masked cov tiles,
    list of (BPC, D, D)."""
    nc = _get_kernel()
    in_maps = [
        {"x": np.ascontiguousarray(x_full[c * BPC:(c + 1) * BPC])}
        for c in range(NCORES)
    ]
    res = run_bass_kernel_spmd(nc, in_maps, core_ids=list(range(NCORES)))
    return [res.results[c]["out"] for c in range(NCORES)]


def postprocess(all_cov: np.ndarray) -> np.ndarray:
    """(B, D, D) masked cov tiles -> (B, D_OUT) globally sorted triu."""
    iu, ju = np.triu_indices(D)
    tri = all_cov[:, iu, ju]  # (B, D_OUT)
    return np.sort(tri.reshape(-1)).reshape(B, D_OUT).astype(np.float32)


def kernel(x: np.ndarray) -> np.ndarray:
    x = np.asarray(x, dtype=np.float32)
    covs = run_device(x)  # 8 x (BPC, D, D), lower tri = BIG
    all_cov = np.concatenate(covs, axis=0)  # (B, D, D)
    return postprocess(all_cov)


if __name__ == "__main__":
    rng = np.random.default_rng(0)
    xt = rng.standard_normal((B, N, D), dtype=np.float32)
    out = kernel(xt)
    print("kernel out shape:", out.shape, out.dtype)
